# revision 90
# baseline (speedup 1.0000x reference)
"""BarrierNet Trainium2 kernel: MLP + batched closed-form 2D QP solve.

Data-parallel across 8 NeuronCores: each core handles 8192 rows of the
65536-row batch.  Per core:
  - MLP (8->256->{128,128}->2,2) on TensorEngine, feature-major layout,
    16 chunks of 512 rows; xT loaded via strided DMA from DRAM.
  - MLP outputs (p31, x32pre) transposed to row-major layout via PE.
  - QP candidate enumeration (1 + 9 + 36 KKT candidates, 9 constraints)
    as packed row-layout elementwise ops on Vector/Scalar/GpSimd engines.
    Row r = g*128 + p lives at [partition p, free-group g]; constraint /
    pair / candidate indices pack along the free axis.
  - Feasibility: per-constraint slack products split between GPSIMD
    (candidates k>=KD) and DVE, compares + mask updates on DVE (f32,
    matching the reference's rounding and tolerance semantics).
  - MLP matmuls run in float32r (full-rate fp32 on the PE array).
  - The core batch is processed in NH=2 halves so stages of one half
    overlap the other half's on otherwise-idle engines.

Self-contained: hardcodes shapes; builds + compiles the Bass graph once
(cached), runs via run_bass_kernel_spmd on cores 0..7.
"""
import math
from contextlib import ExitStack

import numpy as np

import concourse.bass as bass
import concourse.tile as tile
from concourse import bacc, mybir
from concourse.bass_utils import run_bass_kernel_spmd
from concourse.masks import make_identity

FP = mybir.dt.float32
FR = mybir.dt.float32r
BF = mybir.dt.bfloat16
AF = mybir.ActivationFunctionType
OP = mybir.AluOpType

P = 128          # partitions
NCORE = 8
BTOT = 65536
B = BTOT // NCORE    # rows per core = 8192
G = B // P           # row groups per core = 64
NH = 2               # halves (pipeline MLP of h1 over QP of h0)
GW = G // NH         # groups per half = 32
NCH = 16             # MLP chunks per core
CH = B // NCH        # rows per chunk = 512
GPC = CH // P        # groups per chunk = 4

NOBS = 8
NCON = 9             # 8 obstacles + opponent
NPAIR = 16
NCAND = 26           # 1 + 9 + 16 (adjacent-obstacle + obstacle-opponent pairs)

ANG = np.linspace(0.0, 2.0 * np.pi, NOBS, endpoint=False)
CA = [float(np.float32(np.cos(a))) for a in ANG]
SA = [float(np.float32(np.sin(a))) for a in ANG]
R2 = 0.64            # (0.2+0.5+0.1)^2
RO2 = 0.25           # (2*0.2+0.1)^2
BIG = 1.0e30
PI = math.pi

INPUT_SPECS = {
    "x": (B, 8), "mean": (8,), "std": (8,),
    "W1": (256, 8), "b1": (256,),
    "W21": (128, 256), "b21": (128,),
    "W31": (2, 128), "b31": (2,),
    "W22": (128, 256), "b22": (128,),
    "W32": (2, 128), "b32": (2,),
}


def ap_bcast(t_ap: bass.AP, reps: int) -> bass.AP:
    """[128, F] AP -> [128, reps, F] AP broadcast along a stride-0 middle dim."""
    ap = [list(d) for d in t_ap.ap]
    assert len(ap) == 2, ap
    return bass.AP(t_ap.tensor, t_ap.offset, [ap[0], [0, reps], ap[1]])


def build_graph():
    nc = bacc.Bacc(
        "TRN2",
        target_bir_lowering=False,
        debug=False,
        enable_asserts=False,
        num_devices=NCORE,
    )
    ins = {}
    for name, shape in INPUT_SPECS.items():
        ins[name] = nc.dram_tensor(name, list(shape), FP, kind="ExternalInput").ap()
    out_ap = nc.dram_tensor("out", [B, 2], FP, kind="ExternalOutput").ap()

    with tile.TileContext(nc) as tc:
        with ExitStack() as ctx:
            _build(ctx, tc, out_ap, ins)
    nc.compile()
    return nc


def _prep_weights(ctx, tc, ins):
    """Load + transpose weights into lhsT form; returns dict of tiles."""
    nc = tc.nc
    S = nc.scalar
    V = nc.vector
    GP = nc.gpsimd
    T = nc.tensor

    consts = ctx.enter_context(tc.tile_pool(name="consts", bufs=1))
    ident = consts.tile([P, P], FP)
    make_identity(nc, ident[:])

    wpool = ctx.enter_context(tc.tile_pool(name="wpool", bufs=1))
    psum_w_ctx = ExitStack()
    psum_w = psum_w_ctx.enter_context(tc.tile_pool(name="psum_w", bufs=1, space="PSUM"))

    W = {"ident": ident}

    # W1 [256, 8]: lhsT chunks W1T[a][8, 128] = W1[a*128:(a+1)*128, :].T
    w1_sb = wpool.tile([P, 16], FP)
    nc.sync.dma_start(out=w1_sb[:, 0:8], in_=ins["W1"][0:128, :])
    nc.sync.dma_start(out=w1_sb[:, 8:16], in_=ins["W1"][128:256, :])
    W1T = wpool.tile([8, 256], FR)
    pw = psum_w.tile([8, 256], FP)
    T.transpose(pw[:, 0:128], w1_sb[:, 0:8], ident[:])
    T.transpose(pw[:, 128:256], w1_sb[:, 8:16], ident[:])
    S.copy(W1T[:], pw[:])
    W["W1T"] = W1T

    # W21/W22 [128, 256]: lhsT chunks [128,128] = W2x[:, a*128:(a+1)*128].T
    for name in ("W21", "W22"):
        dst = wpool.tile([P, 256], FR, name=name + "T")
        w_sb = wpool.tile([P, 256], FP, tag="w2_stage", name="w2_stage")
        nc.sync.dma_start(out=w_sb[:], in_=ins[name][:, :])
        pw2 = psum_w.tile([P, 256], FP, tag="pw2", name="pw2")
        T.transpose(pw2[:, 0:128], w_sb[:, 0:128], ident[:])
        T.transpose(pw2[:, 128:256], w_sb[:, 128:256], ident[:])
        S.copy(dst[:], pw2[:])
        W[name + "T"] = dst

    # W31/W32 [2, 128] -> zero-padded lhsT [128, 4] so both final matmuls
    # can accumulate into one [4, CH] PSUM tile at base partition 0.
    W31z = wpool.tile([P, 4], FR)
    W32z = wpool.tile([P, 4], FR)
    w3f = wpool.tile([P, 8], FP)
    GP.memset(w3f[:], 0.0)
    w3_sb = wpool.tile([2, 256], FP)
    nc.sync.dma_start(out=w3_sb[:, 0:128], in_=ins["W31"][:, :])
    nc.sync.dma_start(out=w3_sb[:, 128:256], in_=ins["W32"][:, :])
    pw3 = psum_w.tile([P, 4], FP)
    T.transpose(pw3[:, 0:2], w3_sb[:, 0:128], ident[0:2, 0:2])
    T.transpose(pw3[:, 2:4], w3_sb[:, 128:256], ident[0:2, 0:2])
    V.tensor_copy(w3f[:, 0:2], pw3[:, 0:2])
    V.tensor_copy(w3f[:, 6:8], pw3[:, 2:4])
    S.copy(W31z[:], w3f[:, 0:4])
    S.copy(W32z[:], w3f[:, 4:8])
    W["W31z"] = W31z
    W["W32z"] = W32z

    # biases as column tiles
    b1_sb = wpool.tile([P, 2], FP)
    nc.sync.dma_start(out=b1_sb[:], in_=bass.AP(ins["b1"].tensor, 0, [[1, P], [P, 2]]))
    b21_sb = wpool.tile([P, 1], FP)
    nc.sync.dma_start(out=b21_sb[:], in_=bass.AP(ins["b21"].tensor, 0, [[1, P], [1, 1]]))
    b22_sb = wpool.tile([P, 1], FP)
    nc.sync.dma_start(out=b22_sb[:], in_=bass.AP(ins["b22"].tensor, 0, [[1, P], [1, 1]]))
    std_sb = wpool.tile([P, 8], FP)
    nc.sync.dma_start(out=std_sb[:], in_=bass.AP(ins["std"].tensor, 0, [[0, P], [1, 8]]))
    mean_sb = wpool.tile([P, 8], FP)
    nc.sync.dma_start(out=mean_sb[:], in_=bass.AP(ins["mean"].tensor, 0, [[0, P], [1, 8]]))
    b31_sb = wpool.tile([P, 2], FP)
    nc.sync.dma_start(out=b31_sb[:], in_=bass.AP(ins["b31"].tensor, 0, [[0, P], [1, 2]]))
    b32_sb = wpool.tile([P, 2], FP)
    nc.sync.dma_start(out=b32_sb[:], in_=bass.AP(ins["b32"].tensor, 0, [[0, P], [1, 2]]))
    zero_c = wpool.tile([P, 2], FP)
    GP.memset(zero_c[:, 0:1], 0.0)
    GP.memset(zero_c[:, 1:2], 1.0e25)
    W.update(b1=b1_sb, b21=b21_sb, b22=b22_sb, std=std_sb, mean=mean_sb,
             b31=b31_sb, b32=b32_sb, zh=zero_c)
    psum_w_ctx.close()
    return W


def _mlp_half(ctx, tc, h, ins, W, mpool, ppool, QR_h):
    """MLP for half h: writes QR_h [128, GW*4] (q = p31x,p31y,x32a,x32b)."""
    nc = tc.nc
    V = nc.vector
    S = nc.scalar
    T = nc.tensor
    x_dram = ins["x"]
    ident = W["ident"]

    ch0 = h * (NCH // NH)
    for nci in range(NCH // NH):
        n = ch0 + nci
        r0 = n * CH
        # xT [8, CH] straight from DRAM (per-feature strided read), then
        # round to fp32r for the PE
        xTs = mpool.tile([8, CH], FP, tag="xTs", name="xTs")
        src = bass.AP(x_dram.tensor, r0 * 8, [[1, 8], [8, CH]])
        nc.sync.dma_start(out=xTs[:], in_=src)
        xT = mpool.tile([8, CH], FR, tag="xT", name="xT")
        if nci % 2 == 0:
            S.copy(xT[:], xTs[:])
        else:
            V.tensor_copy(xT[:], xTs[:])

        ph1a = ppool.tile([P, CH], FP, tag="ph1a", name="ph1a", bufs=2)
        T.matmul(ph1a[:], W["W1T"][:, 0:128], xT[:])
        ph1b = ppool.tile([P, CH], FP, tag="ph1b", name="ph1b", bufs=2)
        T.matmul(ph1b[:], W["W1T"][:, 128:256], xT[:])
        A1a = mpool.tile([P, CH], FR, tag="A1a", name="A1a")
        A1b = mpool.tile([P, CH], FR, tag="A1b", name="A1b")
        if nci % 2 == 0:
            V.tensor_scalar(A1a[:], ph1a[:], W["b1"][:, 0:1], 0.0, OP.add, OP.max)
            S.activation(A1b[:], ph1b[:], AF.Relu, bias=W["b1"][:, 1:2], scale=1.0)
        else:
            S.activation(A1a[:], ph1a[:], AF.Relu, bias=W["b1"][:, 0:1], scale=1.0)
            V.tensor_scalar(A1b[:], ph1b[:], W["b1"][:, 1:2], 0.0, OP.add, OP.max)

        pa2 = ppool.tile([P, CH], FP, tag="pa2", name="pa2")
        T.matmul(pa2[:], W["W21T"][:, 0:128], A1a[:], start=True, stop=False)
        T.matmul(pa2[:], W["W21T"][:, 128:256], A1b[:], start=False, stop=True)
        A2 = mpool.tile([P, CH], FR, tag="A2", name="A2")
        if nci % 2 == 0:
            V.tensor_scalar(A2[:], pa2[:], W["b21"][:, 0:1], 0.0, OP.add, OP.max)
        else:
            S.activation(A2[:], pa2[:], AF.Relu, bias=W["b21"][:, 0:1], scale=1.0)

        ps2 = ppool.tile([P, CH], FP, tag="ps2", name="ps2")
        T.matmul(ps2[:], W["W22T"][:, 0:128], A1a[:], start=True, stop=False)
        T.matmul(ps2[:], W["W22T"][:, 128:256], A1b[:], start=False, stop=True)
        S2h = mpool.tile([P, CH], FR, tag="S2h", name="S2h")
        S.activation(S2h[:], ps2[:], AF.Relu, bias=W["b22"][:, 0:1], scale=1.0)

        pp = ppool.tile([4, CH], FP, tag="pp", name="pp")
        T.matmul(pp[:], W["W31z"][:], A2[:], start=True, stop=False)
        T.matmul(pp[:], W["W32z"][:], S2h[:], start=False, stop=True)
        qt4 = mpool.tile([4, CH], FP, tag="qt4", name="qt4")
        V.tensor_copy(qt4[:], pp[:])

        pqr = ppool.tile([P, 4 * GPC], FP, tag="pqr", name="pqr")
        for i in range(GPC):
            T.transpose(pqr[:, i * 4:(i + 1) * 4],
                        qt4[:, i * P:(i + 1) * P], ident[0:4, 0:4])
        S.copy(QR_h[:, nci * 4 * GPC:(nci + 1) * 4 * GPC], pqr[:])


def _qp_head(tc, h, ins, W, QR_h, rp):
    """QP phase for half h through the z1 candidates; returns state dict."""
    nc = tc.nc
    V = nc.vector
    S = nc.scalar
    GP = nc.gpsimd

    def rt(tag, w=GW, dt=FP):
        return rp.tile([P, w], dt, tag=tag, name=tag)

    # x features, row layout, straight from DRAM (per-feature strided)
    Xr = rp.tile([P, GW * 8], FP, tag="Xr", name="Xr")
    src = bass.AP(ins["x"].tensor, h * GW * P * 8, [[8, P], [8 * P, GW], [1, 8]])
    nc.sync.dma_start(out=Xr[:], in_=src)

    def xs(c):
        return bass.AP(Xr[:].tensor, Xr[:].offset + c, [Xr[:].ap[0], [8, GW]])

    x0 = []
    for c in range(8):
        t = rt(f"x0_{c}")
        V.tensor_scalar(t[:], xs(c), W["std"][:, c:c + 1], W["mean"][:, c:c + 1],
                        OP.mult, OP.add)
        x0.append(t)
    px, py, th, v, ox, oy, oth, ov = x0

    p31x = rt("p31x"); p31y = rt("p31y"); sg0 = rt("sg0"); sg1 = rt("sg1")

    def qr_slice(q):
        return bass.AP(QR_h[:].tensor, QR_h[:].offset + q, [QR_h[:].ap[0], [4, GW]])

    V.tensor_scalar(p31x[:], qr_slice(0), W["b31"][:, 0:1], None, OP.add)
    V.tensor_scalar(p31y[:], qr_slice(1), W["b31"][:, 1:2], None, OP.add)
    S.activation(sg0[:], qr_slice(2), AF.Sigmoid, bias=W["b32"][:, 0:1], scale=1.0)
    S.activation(sg1[:], qr_slice(3), AF.Sigmoid, bias=W["b32"][:, 1:2], scale=1.0)

    # trig
    st = rt("st"); ct = rt("ct"); so = rt("so"); co = rt("co")
    wr = rt("wrap_tmp")
    V.add_range_wrap(wr[:], th[:], 0.0, PI, 2 * PI)
    S.activation(st[:], wr[:], AF.Sin)
    wr2 = rt("wrap_tmp2")
    V.add_range_wrap(wr2[:], th[:], PI / 2, PI, 2 * PI)
    S.activation(ct[:], wr2[:], AF.Sin)
    wr3 = rt("wrap_tmp3")
    V.add_range_wrap(wr3[:], oth[:], 0.0, PI, 2 * PI)
    S.activation(so[:], wr3[:], AF.Sin)
    wr4 = rt("wrap_tmp4")
    V.add_range_wrap(wr4[:], oth[:], PI / 2, PI, 2 * PI)
    S.activation(co[:], wr4[:], AF.Sin)

    vs2 = rt("vs2"); vc2 = rt("vc2"); ct2 = rt("ct2"); st2 = rt("st2")
    V.scalar_tensor_tensor(vs2[:], v[:], 2.0, st[:], OP.mult, OP.mult)
    V.scalar_tensor_tensor(vc2[:], v[:], 2.0, ct[:], OP.mult, OP.mult)
    S.mul(ct2[:], ct[:], 2.0)
    S.mul(st2[:], st[:], 2.0)

    # coefficients
    axc = rt("axc"); bxc = rt("bxc"); cxc = rt("cxc")
    ayn = rt("ayn"); byc = rt("byc"); cyc = rt("cyc")

    def tmp():
        return rp.tile([P, GW], FP, tag="ctmp", name="ctmp", bufs=10)

    def mulpair(out, a1, b1, a2, b2, op=OP.subtract):
        u = tmp(); w = tmp()
        V.tensor_mul(u[:], a1[:], b1[:])
        GP.tensor_mul(w[:], a2[:], b2[:])
        V.tensor_tensor(out[:], u[:], w[:], op)

    mulpair(axc, px, vs2, py, vc2, OP.subtract)
    S.mul(bxc[:], vs2[:], -10.0)
    S.mul(cxc[:], vc2[:], 10.0)
    mulpair(ayn, px, ct2, py, st2, OP.add)       # = -ay
    S.mul(byc[:], ct2[:], 10.0)
    S.mul(cyc[:], st2[:], 10.0)

    v2t = rt("v2t"); d0 = rt("d0"); d1 = rt("d1"); d2 = rt("d2")
    e0 = rt("e0"); e1 = rt("e1"); e2 = rt("e2")
    GP.tensor_mul(v2t[:], v[:], v[:])
    mulpair(d0, px, vc2, py, vs2, OP.add)
    S.mul(d1[:], vc2[:], -10.0)
    S.mul(d2[:], vs2[:], -10.0)
    mulpair(e0, px, px, py, py, OP.add)
    V.tensor_scalar(e0[:], e0[:], 100.0 - R2, None, OP.add)
    S.mul(e1[:], px[:], -20.0)
    S.mul(e2[:], py[:], -20.0)

    S4 = rt("S4"); P16 = rt("P16")
    tS = tmp(); tP = tmp()
    V.scalar_tensor_tensor(tS[:], sg0[:], 1.0, sg1[:], OP.mult, OP.add)
    S.mul(S4[:], tS[:], 4.0)
    GP.tensor_mul(tP[:], sg0[:], sg1[:])
    S.mul(P16[:], tP[:], 16.0)

    f0 = rt("f0"); f1 = rt("f1"); f2 = rt("f2")
    tf = tmp()
    mulpair(tf, S4, d0, P16, e0, OP.add)
    V.scalar_tensor_tensor(f0[:], v2t[:], 2.0, tf[:], OP.mult, OP.add)
    mulpair(f1, S4, d1, P16, e1, OP.add)
    mulpair(f2, S4, d2, P16, e2, OP.add)

    # constraints
    Gx = rp.tile([P, NCON * GW], FP, tag="Gx", name="Gx")
    Gy = rp.tile([P, NCON * GW], FP, tag="Gy", name="Gy")
    ht = rp.tile([P, NCON * GW], FP, tag="ht", name="ht")

    def sl(tile_, c, n=1):
        return tile_[:, c * GW:(c + n) * GW]

    for c in range(NOBS):
        ta = tmp(); tb = tmp(); tcm = tmp()
        V.scalar_tensor_tensor(ta[:], bxc[:], CA[c], axc[:], OP.mult, OP.add)
        V.scalar_tensor_tensor(sl(Gx, c), cxc[:], SA[c], ta[:], OP.mult, OP.add)
        V.scalar_tensor_tensor(tb[:], byc[:], CA[c], ayn[:], OP.mult, OP.subtract)
        V.scalar_tensor_tensor(sl(Gy, c), cyc[:], SA[c], tb[:], OP.mult, OP.add)
        V.scalar_tensor_tensor(tcm[:], f1[:], CA[c], f0[:], OP.mult, OP.add)
        V.scalar_tensor_tensor(sl(ht, c), f2[:], SA[c], tcm[:], OP.mult, OP.add)

    # opponent constraint (c=8)
    dxo = rt("dxo"); dyo = rt("dyo")
    V.tensor_sub(dxo[:], px[:], ox[:])
    GP.tensor_sub(dyo[:], py[:], oy[:])
    g8 = tmp()
    mulpair(g8, dxo, vs2, dyo, vc2, OP.subtract)
    V.tensor_copy(sl(Gx, 8), g8[:])
    g8y = tmp()
    mulpair(g8y, dxo, ct2, dyo, st2, OP.add)
    S.mul(sl(Gy, 8), g8y[:], -1.0)
    cd = rt("cosdiff"); u1t = rt("u1t"); u2t = rt("u2t")
    mulpair(cd, ct, co, st, so, OP.add)
    tvo = tmp(); tvv = tmp()
    GP.tensor_mul(tvo[:], v[:], ov[:])
    V.tensor_mul(tvo[:], tvo[:], cd[:])
    GP.tensor_mul(tvv[:], ov[:], ov[:])
    V.scalar_tensor_tensor(tvo[:], tvo[:], 2.0, tvv[:], OP.mult, OP.add)
    V.tensor_add(tvo[:], tvo[:], v2t[:])
    lf2o = rt("lf2o")
    S.mul(lf2o[:], tvo[:], 2.0)
    V.tensor_mul(u1t[:], ov[:], co[:])
    V.scalar_tensor_tensor(u1t[:], vc2[:], 0.5, u1t[:], OP.mult, OP.subtract)
    GP.tensor_mul(u2t[:], ov[:], so[:])
    V.scalar_tensor_tensor(u2t[:], vs2[:], 0.5, u2t[:], OP.mult, OP.subtract)
    bdo = rt("bdo")
    tb1 = tmp()
    mulpair(tb1, dxo, u1t, dyo, u2t, OP.add)
    S.mul(bdo[:], tb1[:], 2.0)
    baro = rt("baro")
    tb2 = tmp()
    mulpair(tb2, dxo, dxo, dyo, dyo, OP.add)
    V.tensor_scalar(baro[:], tb2[:], -RO2, None, OP.add)
    th8 = tmp()
    mulpair(th8, S4, bdo, P16, baro, OP.add)
    V.tensor_add(sl(ht, 8), th8[:], lf2o[:])

    # hpt = h + 1e-6*(1+|h|)
    hpt = rp.tile([P, NCON * GW], FP, tag="hpt", name="hpt")
    habs = rp.tile([P, NCON * GW], FP, tag="habs", name="habs")
    S.activation(habs[:], ht[:], AF.Abs)
    V.affine_then_add(hpt[:], habs[:], ht[:], 1e-6, 1e-6)

    # candidates
    CW = NCAND * GW
    Zx = rp.tile([P, CW], FP, tag="Zx", name="Zx")
    Zy = rp.tile([P, CW], FP, tag="Zy", name="Zy")
    VAL = rp.tile([P, CW], FP, tag="VAL", name="VAL")

    S.mul(Zx[:, 0:GW], p31x[:], -1.0)
    S.mul(Zy[:, 0:GW], p31y[:], -1.0)
    GP.memset(VAL[:, 0:GW], 1.0)

    # k=1..9: single active constraint
    z1_ctx = ExitStack()
    z1pool = z1_ctx.enter_context(tc.tile_pool(name=f"z1pool{h}", bufs=1))
    gg = z1pool.tile([P, NCON * GW], FP, tag="gg", name="gg")
    rgg = z1pool.tile([P, NCON * GW], FP, tag="rgg", name="rgg")
    gtmp = z1pool.tile([P, NCON * GW], FP, tag="gtmp", name="gtmp")
    lam1 = z1pool.tile([P, NCON * GW], FP, tag="lam1", name="lam1")
    V.tensor_mul(gg[:], Gx[:], Gx[:])
    GP.tensor_mul(gtmp[:], Gy[:], Gy[:])
    V.tensor_add(gg[:], gg[:], gtmp[:])
    V.tensor_scalar(gg[:], gg[:], 1e-12, None, OP.add)
    V.reciprocal_approx_accurate(rgg[:], gg[:], gtmp[:])
    px31_b9 = ap_bcast(p31x[:], NCON)
    py31_b9 = ap_bcast(p31y[:], NCON)
    GP.tensor_tensor(gg[:], Gx[:], px31_b9, OP.mult)
    V.tensor_tensor(gtmp[:], Gy[:], py31_b9, OP.mult)
    V.tensor_add(gg[:], gg[:], gtmp[:])
    V.tensor_add(gg[:], gg[:], ht[:])              # Gp + h
    V.scalar_tensor_tensor(lam1[:], gg[:], -1.0, rgg[:], OP.mult, OP.mult)
    V.scalar_tensor_tensor(gtmp[:], lam1[:], -1.0, Gx[:], OP.mult, OP.mult)
    V.tensor_tensor(Zx[:, GW:10 * GW], gtmp[:], px31_b9, OP.subtract)
    GP.tensor_tensor(gtmp[:], lam1[:], Gy[:], OP.mult)
    V.scalar_tensor_tensor(Zy[:, GW:10 * GW], gtmp[:], -1.0, py31_b9,
                           OP.mult, OP.subtract)
    V.tensor_scalar(VAL[:, GW:10 * GW], lam1[:], -1e-8, None, OP.is_ge)
    z1_ctx.close()
    return dict(rp=rp, Gx=Gx, Gy=Gy, ht=ht, hpt=hpt, Zx=Zx, Zy=Zy, VAL=VAL,
                p31x=p31x, p31y=p31y, sl=sl, rt=rt, W=W)


def _qp_tail(tc, h, st):
    """Pairs + objective + feasibility; leaves objm in st (fpool open)."""
    nc = tc.nc
    V = nc.vector
    S = nc.scalar
    GP = nc.gpsimd
    rp = st["rp"]; rt = st["rt"]; sl = st["sl"]
    Gx = st["Gx"]; Gy = st["Gy"]; ht = st["ht"]; hpt = st["hpt"]
    Zx = st["Zx"]; Zy = st["Zy"]; VAL = st["VAL"]
    p31x = st["p31x"]; p31y = st["p31y"]
    CW = NCAND * GW

    # Pairs k=10..25: only the geometrically reachable vertices --
    # 8 adjacent-obstacle pairs (i, (i+1)%8) and 8 obstacle-opponent pairs
    # (i, 8).  Non-adjacent obstacle pairs can never be jointly active for
    # this problem's geometry (obstacles on a radius-10 ring, agents near
    # the origin); verified exhaustively against the full 46-candidate
    # enumeration on the reference dataset (0 differing rows).
    # The i-side of every pair tensor is the obstacle block repeated 2x
    # (stride-0 AP, no copies); the j-side needs 3 small copies per tensor.
    with tc.tile_pool(name=f"pairs{h}", bufs=1) as pp2:
        PW = NPAIR * GW
        OBW = NOBS * GW

        def rep2(tile_):
            t = tile_[:]
            return bass.AP(t.tensor, t.offset, [t.ap[0], [0, 2], [1, OBW]])

        GiX = rep2(Gx)
        GiY = rep2(Gy)
        hi = rep2(ht)
        GjX = pp2.tile([P, PW], FP, tag="GjX", name="GjX")
        GjY = pp2.tile([P, PW], FP, tag="GjY", name="GjY")
        hj = pp2.tile([P, PW], FP, tag="hj", name="hj")
        # block 1 (pair k=10..17): obstacle-opponent pairs (j = opp);
        # block 2 (pair k=18..25): adjacent-obstacle pairs (j = (i+1)%8).
        # This order makes candidates k=0..17 (everything that obstacle
        # constraints can ever reject) one contiguous range for feasibility.
        for tsrc, tdst, eng in ((Gx, GjX, V), (Gy, GjY, GP), (ht, hj, S)):
            cp = eng.tensor_copy if eng is not S else eng.copy
            cp(tdst[:, 0:8 * GW], ap_bcast(sl(tsrc, 8), 8))
            cp(tdst[:, 8 * GW:15 * GW], tsrc[:, GW:8 * GW])
            cp(tdst[:, 15 * GW:16 * GW], tsrc[:, 0:GW])

        det_ok = pp2.tile([P, PW], FP, tag="det_ok", name="det_ok")
        rds = pp2.tile([P, PW], FP, tag="rds", name="rds")
        pA = pp2.tile([P, PW], FP, tag="pA", name="pA", bufs=2)
        pB = pp2.tile([P, PW], FP, tag="pB", name="pB", bufs=2)
        pC = pp2.tile([P, PW], FP, tag="pC", name="pC", bufs=2)
        pD = pp2.tile([P, PW], FP, tag="pD", name="pD", bufs=2)
        rx = pp2.tile([P, PW], FP, tag="rx", name="rx")
        ry = pp2.tile([P, PW], FP, tag="ry", name="ry")
        # det -> ds -> rds
        GP.tensor_mul(pA[:], GiX, GjY[:])
        V.scalar_tensor_tensor(pB[:], GiY, -1.0, GjX[:], OP.mult, OP.mult)
        V.tensor_add(pB[:], pA[:], pB[:])          # det
        S.activation(pA[:], pB[:], AF.Abs)
        V.tensor_scalar(det_ok[:], pA[:], 1e-9, None, OP.is_gt)
        V.tensor_scalar(pA[:], pB[:], -1.0, None, OP.add)
        V.tensor_mul(pA[:], pA[:], det_ok[:])
        V.tensor_scalar(pA[:], pA[:], 1.0, None, OP.add)   # ds
        V.reciprocal_approx_accurate(rds[:], pA[:], pC[:])
        # zx = (hi*GjY - hj*GiY) * rds ; zy = (GiX*hj - GjX*hi) * rds
        zx_s = Zx[:, 10 * GW:NCAND * GW]
        zy_s = Zy[:, 10 * GW:NCAND * GW]
        GP.tensor_mul(pA[:], hi, GjY[:])
        V.scalar_tensor_tensor(pB[:], hj[:], -1.0, GiY, OP.mult, OP.mult)
        V.tensor_add(pB[:], pA[:], pB[:])
        GP.tensor_mul(zx_s, pB[:], rds[:])
        V.tensor_mul(pC[:], GiX, hj[:])
        V.scalar_tensor_tensor(pD[:], hi, -1.0, GjX[:], OP.mult, OP.mult)
        V.tensor_add(pD[:], pC[:], pD[:])
        GP.tensor_mul(zy_s, pD[:], rds[:])
        # rx = -zx - p31x ; ry = -zy - p31y
        px31_b36 = ap_bcast(p31x[:], NPAIR)
        py31_b36 = ap_bcast(p31y[:], NPAIR)
        V.scalar_tensor_tensor(rx[:], zx_s, -1.0, px31_b36, OP.mult, OP.subtract)
        V.scalar_tensor_tensor(ry[:], zy_s, -1.0, py31_b36, OP.mult, OP.subtract)
        # lam_i = (GjY*rx - GjX*ry)*rds ; lam_j = (GiX*ry - GiY*rx)*rds
        GP.tensor_mul(pA[:], GjY[:], rx[:])
        V.scalar_tensor_tensor(pB[:], GjX[:], -1.0, ry[:], OP.mult, OP.mult)
        V.tensor_add(pB[:], pA[:], pB[:])
        GP.tensor_mul(pB[:], pB[:], rds[:])        # lam_i
        V.tensor_scalar(pB[:], pB[:], -1e-8, None, OP.is_ge)
        GP.tensor_mul(pB[:], pB[:], det_ok[:])
        V.tensor_mul(pC[:], GiX, ry[:])
        V.scalar_tensor_tensor(pD[:], GiY, -1.0, rx[:], OP.mult, OP.mult)
        V.tensor_add(pD[:], pC[:], pD[:])
        GP.tensor_mul(pD[:], pD[:], rds[:])        # lam_j
        V.tensor_scalar(pD[:], pD[:], -1e-8, None, OP.is_ge)
        V.tensor_mul(VAL[:, 10 * GW:NCAND * GW], pB[:], pD[:])

    # objective (f32)
    f_ctx = ExitStack()
    fpool = f_ctx.enter_context(tc.tile_pool(name=f"feas{h}", bufs=1))
    obj = fpool.tile([P, CW], FP, tag="obj", name="obj")
    m1 = fpool.tile([P, CW], FP, tag="m1", name="m1")
    m2 = fpool.tile([P, CW], FP, tag="m2", name="m2")
    px2 = rt("px2t"); py2 = rt("py2t")
    S.mul(px2[:], p31x[:], 2.0)
    S.mul(py2[:], p31y[:], 2.0)
    V.tensor_tensor(m1[:], Zx[:], ap_bcast(px2[:], NCAND), OP.add)
    V.scalar_tensor_tensor(m1[:], Zx[:], 0.5, m1[:], OP.mult, OP.mult)
    GP.tensor_tensor(m2[:], Zy[:], ap_bcast(py2[:], NCAND), OP.add)
    V.scalar_tensor_tensor(m2[:], Zy[:], 0.5, m2[:], OP.mult, OP.mult)
    V.tensor_add(obj[:], m1[:], m2[:])

    # ---- feasibility (f32, matches reference rounding/tolerance) ----
    # candidate range split: DVE handles k<KD, Pool handles k>=KD (its
    # ~1.9x slower per-element rate balances against DVE's 30/16 share)
    # Sparse feasibility check matrix (verified exact vs the full matrix on
    # the reference dataset): obstacle constraints (c<8) can only ever
    # reject candidates that do not already include an obstacle constraint
    # in their active set -- z0, the z1 projections, and the opponent pairs
    # (k = 0..17, contiguous).  Adjacent-obstacle pairs (k=18..25) are
    # checked only against the opponent constraint (c=8), which checks all
    # candidates.  Pool computes the slack products for part of each range.
    KD = 9
    NKD = KD * GW
    N18 = 18 * GW
    for c in range(NCON):
        wid = NCAND if c == 8 else 18
        nkd = NKD if c == 8 else 8 * GW
        nw = wid * GW
        fm1 = fpool.tile([P, CW], FP, tag="fm1", name="fm1", bufs=3)
        fm2 = fpool.tile([P, CW], FP, tag="fm2", name="fm2", bufs=3)
        fm3 = fpool.tile([P, CW], FP, tag="fm3", name="fm3", bufs=3)
        va = fm1[:, 0:nkd]; vb = fm1[:, nkd:nw]
        wa = fm2[:, 0:nkd]; wb = fm2[:, nkd:nw]
        ua = fm3[:, 0:nkd]; ub = fm3[:, nkd:nw]
        kda = nkd // GW
        gx_a = ap_bcast(sl(Gx, c), kda); gx_bb = ap_bcast(sl(Gx, c), wid - kda)
        gy_a = ap_bcast(sl(Gy, c), kda); gy_bb = ap_bcast(sl(Gy, c), wid - kda)
        hp_a = ap_bcast(sl(hpt, c), kda); hp_bb = ap_bcast(sl(hpt, c), wid - kda)
        GP.tensor_tensor(vb, Zx[:, nkd:nw], gx_bb, OP.mult)
        GP.tensor_tensor(wb, Zy[:, nkd:nw], gy_bb, OP.mult)
        GP.tensor_tensor(wb, vb, wb, OP.add)
        V.tensor_tensor(va, Zx[:, 0:nkd], gx_a, OP.mult)
        V.tensor_tensor(wa, Zy[:, 0:nkd], gy_a, OP.mult)
        V.tensor_add(wa, va, wa)
        V.tensor_tensor(ua, wa, hp_a, OP.is_le)
        V.tensor_mul(VAL[:, 0:nkd], VAL[:, 0:nkd], ua)
        V.tensor_tensor(ub, wb, hp_bb, OP.is_le)
        V.tensor_mul(VAL[:, nkd:nw], VAL[:, nkd:nw], ub)

    # objm = obj*VAL + BIG*(1-VAL)
    objm = m1
    GP.tensor_mul(m2[:], obj[:], VAL[:])
    V.affine_then_add(objm[:], VAL[:], m2[:], -BIG, BIG)
    st["objm"] = objm
    st["f_ctx"] = f_ctx


def _row_argmin(tc, h, out_ap, st):
    nc = tc.nc
    V = nc.vector
    S = nc.scalar
    GP = nc.gpsimd
    rp = st["rp"]; rt = st["rt"]
    Zx = st["Zx"]; Zy = st["Zy"]; objm = st["objm"]

    # argmin chain
    best = rt("best"); bzx = rt("bzx"); bzy = rt("bzy")
    V.tensor_copy(best[:], objm[:, 0:GW])
    S.copy(bzx[:], Zx[:, 0:GW])
    S.copy(bzy[:], Zy[:, 0:GW])
    for k in range(1, NCAND):
        ks = slice(k * GW, (k + 1) * GW)
        lt = rp.tile([P, GW], mybir.dt.int32, tag="lt", name="lt", bufs=2)
        V.tensor_tensor(lt[:], objm[:, ks], best[:], OP.is_lt)
        V.copy_predicated(best[:], lt[:], objm[:, ks])
        V.copy_predicated(bzx[:], lt[:], Zx[:, ks])
        V.copy_predicated(bzy[:], lt[:], Zy[:, ks])

    # output
    obuf = rt("obuf", w=2 * GW)
    ox_ap = bass.AP(obuf[:].tensor, obuf[:].offset, [obuf[:].ap[0], [2, GW]])
    oy_ap = bass.AP(obuf[:].tensor, obuf[:].offset + 1, [obuf[:].ap[0], [2, GW]])
    S.copy(ox_ap, bzx[:])
    GP.tensor_copy(oy_ap, bzy[:])
    dst = bass.AP(out_ap.tensor, h * GW * P * 2, [[2, P], [2 * P, GW], [1, 2]])
    nc.sync.dma_start(out=dst, in_=obuf[:])
    st["f_ctx"].close()


def _build(ctx, tc, out_ap, ins):
    W = _prep_weights(ctx, tc, ins)

    mpool = ctx.enter_context(tc.tile_pool(name="mlp", bufs=3))
    ppool = ctx.enter_context(tc.tile_pool(name="psum_mlp", bufs=1, space="PSUM"))

    persist = ctx.enter_context(tc.tile_pool(name="persistq", bufs=1))
    QRs = [persist.tile([P, GW * 4], FP, tag=f"QR{h}", name=f"QR{h}")
           for h in range(NH)]

    rps = [ctx.enter_context(tc.tile_pool(name=f"rowq{h}", bufs=1))
           for h in range(NH)]
    for h in range(NH):
        _mlp_half(ctx, tc, h, ins, W, mpool, ppool, QRs[h])
    st0 = _qp_head(tc, 0, ins, W, QRs[0], rps[0])
    _qp_tail(tc, 0, st0)
    st1 = _qp_head(tc, 1, ins, W, QRs[1], rps[1])
    _qp_tail(tc, 1, st1)
    _row_argmin(tc, 1, out_ap, st1)
    _row_argmin(tc, 0, out_ap, st0)


_NC_CACHE = None


def _get_graph():
    global _NC_CACHE
    if _NC_CACHE is None:
        _NC_CACHE = build_graph()
    return _NC_CACHE


def kernel(**inputs):
    nc = _get_graph()
    arrs = {k: np.ascontiguousarray(np.asarray(v), dtype=np.float32)
            for k, v in inputs.items() if k in INPUT_SPECS}
    x = arrs["x"]
    in_maps = []
    for c in range(NCORE):
        m = {k: v for k, v in arrs.items() if k != "x"}
        m["x"] = x[c * B:(c + 1) * B]
        in_maps.append(m)
    res = run_bass_kernel_spmd(nc, in_maps, core_ids=list(range(NCORE)))
    outs = [r["out"] for r in res.results]
    return np.concatenate(outs, axis=0)


if __name__ == "__main__":
    nc = build_graph()
    print("graph built + compiled OK")
